# revision 2
# baseline (speedup 1.0000x reference)
"""AdaptiveGroup (octree point grouping) for Trainium2, 8 NeuronCores.

Pipeline:
  1. Device (Bass, SPMD x8): for every point (row), compute KNN scores
     s[j] = 2*x_i.x_j - |x_j|^2 against all 4096 points of its sample via
     PE matmuls (K=4 contraction), and extract the top-8 candidate
     indices of each 1024-column chunk with DVE max8/max_index.
     Row-parallel: core c handles 2048 of the 16384 global rows.
  2. Host: exact refinement of candidates. d2 for candidates is recomputed
     with np.matmul/f32 ops that are bitwise identical to the reference's
     XLA-CPU d2; candidates are sorted by (d2, idx) which reproduces
     jax.lax.top_k exactly. Statistically-rare rows where a chunk may hide
     an extra top-17 member (or any device anomaly) are flagged and
     recomputed exactly in full.
  3. Host: the remaining reference stages (PCA normals via eigh, grid/depth,
     greedy octree budget allocation, RNG sampling + normalization) are
     data-dependent/sequential and tiny; replicated bitwise (jnp-on-CPU for
     the jnp stages, numpy for the eager stages).
"""
import math
import numpy as np

B = 4
N = 4096
NCORES = 8
ROWS_PER_CORE = B * N // NCORES   # 2048
NBLK = ROWS_PER_CORE // 128       # 16
CH = 1024
NCH = N // CH                     # 4
NCAND = NCH * 8                   # 32 candidates per row
NUM_GROUP = 128
GROUP_SIZE = 32
K_NORMALS = 16
DIST_PCT = 50.0
MIN_PTS = 4
# |device score - exact score| bound in d2 units (measured ~5e-6 on HW;
# large safety margin) used by the chunk-insufficiency flag.
EPS_SCORE = 5e-4


# ---------------------------------------------------------------- device ---

def _build_nc():
    import concourse.bacc as bacc
    import concourse.mybir as mybir
    from concourse.tile import TileContext

    nc = bacc.Bacc("TRN2", target_bir_lowering=False, debug=False,
                   num_devices=NCORES)
    lhst = nc.dram_tensor("lhst", [NBLK, 4, 128], mybir.dt.float32,
                          kind="ExternalInput").ap()
    rhs = nc.dram_tensor("rhs", [4, N], mybir.dt.float32,
                         kind="ExternalInput").ap()
    out_idx = nc.dram_tensor("out_idx", [NBLK, 128, NCAND], mybir.dt.uint32,
                             kind="ExternalOutput").ap()
    out_val = nc.dram_tensor("out_val", [NBLK, 128, NCAND], mybir.dt.float32,
                             kind="ExternalOutput").ap()
    with TileContext(nc) as tc:
        with tc.tile_pool(name="const", bufs=1) as cpool, \
             tc.tile_pool(name="w", bufs=3) as wpool, \
             tc.tile_pool(name="res", bufs=3) as rpool, \
             tc.tile_pool(name="psum", bufs=4, space="PSUM") as ppool:
            rhs_sb = cpool.tile([4, N], mybir.dt.float32)
            nc.sync.dma_start(rhs_sb, rhs)
            for blk in range(NBLK):
                lh = wpool.tile([4, 128], mybir.dt.float32, tag="lh")
                nc.sync.dma_start(lh, lhst[blk])
                vals = rpool.tile([128, NCAND], mybir.dt.float32, tag="vals")
                idxs = rpool.tile([128, NCAND], mybir.dt.uint32, tag="idxs")
                for c in range(NCH):
                    ps = ppool.tile([128, CH], mybir.dt.float32, tag="ps")
                    for h in range(CH // 512):
                        nc.tensor.matmul(
                            ps[:, h * 512:(h + 1) * 512], lh,
                            rhs_sb[:, c * CH + h * 512:c * CH + (h + 1) * 512],
                            start=True, stop=True)
                    nc.vector.max(out=vals[:, c * 8:(c + 1) * 8], in_=ps)
                    nc.vector.max_index(idxs[:, c * 8:(c + 1) * 8],
                                        vals[:, c * 8:(c + 1) * 8], ps)
                nc.sync.dma_start(out_idx[blk], idxs)
                nc.sync.dma_start(out_val[blk], vals)
    nc.compile()
    return nc


def _prepare_in_maps(xyz):
    sq = np.sum(xyz * xyz, -1)
    in_maps = []
    for cid in range(NCORES):
        g0 = cid * ROWS_PER_CORE
        b = g0 // N
        r0 = g0 % N
        rows = xyz[b, r0:r0 + ROWS_PER_CORE]
        lhst_np = np.empty((NBLK, 4, 128), np.float32)
        blocks = rows.reshape(NBLK, 128, 3)
        lhst_np[:, 0:3, :] = blocks.transpose(0, 2, 1)
        lhst_np[:, 3, :] = 1.0
        rhs_np = np.empty((4, N), np.float32)
        rhs_np[0:3] = np.float32(2.0) * xyz[b].T
        rhs_np[3] = -sq[b]
        in_maps.append({"lhst": lhst_np, "rhs": rhs_np})
    return in_maps


def run_device(xyz):
    """Returns cand_idx (B,N,NCAND) int64 global column indices."""
    from concourse import bass_utils
    nc = _build_nc()
    in_maps = _prepare_in_maps(xyz)
    res = bass_utils.run_bass_kernel_spmd(nc, in_maps,
                                          core_ids=list(range(NCORES)))
    cand = np.empty((B, N, NCAND), np.int64)
    offs = (np.arange(NCH, dtype=np.int64) * CH).repeat(8)
    for cid in range(NCORES):
        g0 = cid * ROWS_PER_CORE
        b, r0 = g0 // N, g0 % N
        oi = res.results[cid]['out_idx'].reshape(ROWS_PER_CORE, NCAND)
        cand[b, r0:r0 + ROWS_PER_CORE] = oi.astype(np.int64) + offs[None, :]
    return cand


# ------------------------------------------------------------------ host ---

def refine_candidates(x, cand_idx):
    """Exact top-17 (indices+d2), bitwise == jax.lax.top_k(-d2, 17) on the
    reference's f32 d2 as computed by XLA-CPU."""
    Bn, Nn = x.shape[0], x.shape[1]
    sq = np.sum(x * x, -1)

    Y = np.take_along_axis(x[:, None, :, :], cand_idx[..., None], axis=2)
    m = np.matmul(x[:, :, None, :], Y.transpose(0, 1, 3, 2))[:, :, 0, :]
    sqc = np.take_along_axis(sq[:, None, :], cand_idx, axis=2)
    d2c = (sq[:, :, None] + sqc) - np.float32(2.0) * m

    order = np.lexsort((cand_idx, d2c), axis=2)
    idx_s = np.take_along_axis(cand_idx, order, axis=2)
    d_s = np.take_along_axis(d2c, order, axis=2)
    idx17 = idx_s[:, :, :17].astype(np.int32)
    d17 = d_s[:, :, :17].copy()

    flags = np.zeros((Bn, Nn), dtype=bool)
    flags |= idx17[:, :, 0] != np.arange(Nn)[None, :]
    cg = np.sort(cand_idx.reshape(Bn, Nn, NCH, 8), axis=3)
    flags |= (np.diff(cg, axis=3) == 0).any(axis=(2, 3))
    d2g = d2c.reshape(Bn, Nn, NCH, 8)
    chunk8 = d2g.max(axis=3)
    cand17 = d17[:, :, 16]
    flags |= (chunk8 <= cand17[:, :, None] + np.float32(2 * EPS_SCORE)).any(axis=2)

    nflag = int(flags.sum())
    if nflag:
        ar = np.arange(Nn)
        for b in range(Bn):
            rows = np.nonzero(flags[b])[0]
            if rows.size == 0:
                continue
            mrow = np.matmul(x[b][rows], x[b].T)
            d2r = (sq[b][rows][:, None] + sq[b][None, :]) - np.float32(2.0) * mrow
            o = np.lexsort((np.broadcast_to(ar, d2r.shape), d2r), axis=1)[:, :17]
            idx17[b][rows] = o.astype(np.int32)
            d17[b][rows] = np.take_along_axis(d2r, o, axis=1)
    return idx17, d17, nflag


def normals_grid_rest(xyz_np, idx17, d17):
    """Bitwise replica of the reference's post-top_k jnp stages on CPU."""
    import jax
    import jax.numpy as jnp
    cpu = jax.devices('cpu')[0]
    with jax.default_device(cpu):
        xyz = jnp.asarray(xyz_np)
        Bn = xyz.shape[0]
        K = K_NORMALS
        dists = jnp.asarray(d17)
        idx = jnp.asarray(idx17)
        nn_dists = jnp.sqrt(jnp.clip(dists[:, :, 1], 1e-8))
        nbr_idx = idx[:, :, 1:]
        neighbours = xyz[jnp.arange(Bn)[:, None, None], nbr_idx]
        centred = neighbours - xyz[:, :, None, :]
        cov = jnp.einsum('bnki,bnkj->bnij', centred, centred) / K
        _, eigvecs = jnp.linalg.eigh(cov)
        normals = eigvecs[..., :, 0]
        grid_size = jnp.clip(jnp.quantile(nn_dists, DIST_PCT / 100.0, axis=1), 1e-6)
        xyz_min = xyz.min(axis=1)
        bbox_diag = jnp.linalg.norm(xyz.max(axis=1) - xyz_min, axis=-1)
        depth_min = max(1, math.ceil(math.log2(max(NUM_GROUP + 1, 2))))
        raw_depth = jnp.log2(jnp.clip(bbox_diag / grid_size, 1.0))
        depth = jnp.clip(jnp.ceil(raw_depth).astype(jnp.int32), depth_min, 16)
        grid_coord = jnp.floor(
            (xyz - xyz_min[:, None, :]) / grid_size[:, None, None]).astype(jnp.int32)
        return (np.asarray(normals), np.asarray(grid_coord), np.asarray(depth))


def _octree_structure(normals_b, gc_b, max_depth):
    keys = {d: gc_b >> (max_depth - d) for d in range(1, max_depth + 1)}

    def mask_of(d, ix, iy, iz):
        k = keys[d]
        return (k[:, 0] == ix) & (k[:, 1] == iy) & (k[:, 2] == iz)

    def var_count(mask):
        c = int(mask.sum())
        if c < MIN_PTS:
            return 0.0, c
        n = normals_b[mask]
        m = n.mean(0)
        return float(((n - m) ** 2).sum(-1).mean()), c

    frontier = []
    for row in np.unique(keys[1], axis=0):
        ix, iy, iz = [int(v) for v in row]
        msk = mask_of(1, ix, iy, iz)
        v, c = var_count(msk)
        frontier.append(dict(d=1, ix=ix, iy=iy, iz=iz, mask=msk, variance=v,
                             count=c, splittable=(1 < max_depth and c >= MIN_PTS)))
    while len(frontier) < NUM_GROUP:
        spl = [(i, n) for i, n in enumerate(frontier) if n['splittable']]
        if not spl:
            break
        spl.sort(key=lambda t: t[1]['variance'], reverse=True)
        si, sn = spl[0]
        d, ix, iy, iz = sn['d'], sn['ix'], sn['iy'], sn['iz']
        children = []
        for dx in range(2):
            for dy in range(2):
                for dz in range(2):
                    if d + 1 > max_depth:
                        continue
                    cix, ciy, ciz = ix * 2 + dx, iy * 2 + dy, iz * 2 + dz
                    msk = mask_of(d + 1, cix, ciy, ciz)
                    if not msk.any():
                        continue
                    v, c = var_count(msk)
                    children.append(dict(d=d + 1, ix=cix, iy=ciy, iz=ciz,
                                         mask=msk, variance=v, count=c,
                                         splittable=(d + 1 < max_depth and c >= MIN_PTS)))
        frontier.pop(si)
        frontier.extend(children)
    while len(frontier) > NUM_GROUP:
        frontier.sort(key=lambda n: n['variance'])
        frontier.pop(0)
    while len(frontier) < NUM_GROUP:
        frontier.sort(key=lambda n: n['variance'], reverse=True)
        frontier.append(dict(frontier[0]))
    return [n['mask'] for n in frontier]


def group_and_normalize(xyz_np, normals_np, gc_np, depth_np):
    nbhds, ctrs = [], []
    for b in range(xyz_np.shape[0]):
        masks = _octree_structure(normals_np[b], gc_np[b], int(depth_np[b]))
        rng = np.random.default_rng(b)
        nb_b, ct_b = [], []
        for msk in masks:
            pts = xyz_np[b][msk]
            cnt = pts.shape[0]
            centroid = pts.mean(0)
            if cnt >= GROUP_SIZE:
                sel = rng.permutation(cnt)[:GROUP_SIZE]
            else:
                sel = rng.integers(0, cnt, GROUP_SIZE)
            s = pts[sel] - centroid
            scale = max(float(np.linalg.norm(s, axis=-1).max()), 1e-8)
            nb_b.append(s / scale)
            ct_b.append(centroid)
        nbhds.append(np.stack(nb_b))
        ctrs.append(np.stack(ct_b))
    neighborhood = np.stack(nbhds).astype(np.float32)
    center = np.stack(ctrs).astype(np.float32)
    return neighborhood, center


def kernel(xyz):
    xyz = np.ascontiguousarray(np.asarray(xyz), dtype=np.float32)
    assert xyz.shape == (B, N, 3), xyz.shape
    cand = run_device(xyz)
    idx17, d17, _nflag = refine_candidates(xyz, cand)
    normals, gc, depth = normals_grid_rest(xyz, idx17, d17)
    neighborhood, center = group_and_normalize(xyz, normals, gc, depth)
    return neighborhood, center


# revision 3
# speedup vs baseline: 1.5467x; 1.5467x over previous
"""AdaptiveGroup (octree point grouping) for Trainium2, 8 NeuronCores.

Pipeline:
  1. Device (Bass, SPMD x8): for every point (row), compute KNN scores
     s[j] = 2*x_i.x_j - |x_j|^2 against all 4096 points of its sample via
     PE matmuls (K=4 contraction), and extract the top-8 candidate
     indices of each 1024-column chunk with DVE max8/max_index.
     Row-parallel: core c handles 2048 of the 16384 global rows.
  2. Host: exact refinement of candidates. d2 for candidates is recomputed
     with np.matmul/f32 ops that are bitwise identical to the reference's
     XLA-CPU d2; candidates are sorted by (d2, idx) which reproduces
     jax.lax.top_k exactly. Statistically-rare rows where a chunk may hide
     an extra top-17 member (or any device anomaly) are flagged and
     recomputed exactly in full.
  3. Host: the remaining reference stages (PCA normals via eigh, grid/depth,
     greedy octree budget allocation, RNG sampling + normalization) are
     data-dependent/sequential and tiny; replicated bitwise (jnp-on-CPU for
     the jnp stages, numpy for the eager stages).
"""
import math
import numpy as np

B = 4
N = 4096
NCORES = 8
ROWS_PER_CORE = B * N // NCORES   # 2048
NBLK = ROWS_PER_CORE // 128       # 16
NUM_GROUP = 128
GROUP_SIZE = 32
K_NORMALS = 16
DIST_PCT = 50.0
MIN_PTS = 4


# ---------------------------------------------------------------- device ---
# V2 packed-index design. PE matmul (K=5, float32r single-pass) computes
#   p = a*(2 x_i.x_j - sq_j) + (1024 - a*sq_i) = 1024 - a*d2
# so the trusted window d2 in [0, W] maps to p in [512, 1024]. ACT quantizes
# PSUM f32 -> f16 (value grid, step >= 0.5 in-window); GPSIMD/DVE add
# j*2^-11 (column index in the low bits); DVE max8 extracts the top-8
# packed per 1024-column chunk. Host decodes indices from the packed f32.

CH = 1024
NCH = N // CH                     # 4
NCAND = NCH * 8                   # 32 candidates per row
W = 8.0                           # d2 window; max d2_17 on this input is 5.3
A = 512.0 / W
EPS_IOTA = float(2.0 ** -11)
# Selection-noise bound in d2 units: f32r matmul error + f16 quantization +
# iota perturbation, measured <= 1.5 p-units on HW; 2x margin -> 3.0/A.
EPS_SEL = 3.0 / A


def _build_nc():
    import concourse.bacc as bacc
    import concourse.mybir as mybir
    from concourse.tile import TileContext

    nc = bacc.Bacc("TRN2", target_bir_lowering=False, debug=False,
                   num_devices=NCORES)
    lhst = nc.dram_tensor("lhst", [NBLK, 5, 128], mybir.dt.float32,
                          kind="ExternalInput").ap()
    rhs = nc.dram_tensor("rhs", [5, N], mybir.dt.float32,
                         kind="ExternalInput").ap()
    iota = nc.dram_tensor("iota", [128, CH], mybir.dt.float32,
                          kind="ExternalInput").ap()
    out_val = nc.dram_tensor("out_val", [NBLK, 128, NCAND],
                             mybir.dt.float32, kind="ExternalOutput").ap()
    with TileContext(nc) as tc:
        with tc.tile_pool(name="const", bufs=1) as cpool, \
             tc.tile_pool(name="w", bufs=3) as wpool, \
             tc.tile_pool(name="q", bufs=4) as qpool, \
             tc.tile_pool(name="pk", bufs=4) as kpool, \
             tc.tile_pool(name="res", bufs=3) as rpool, \
             tc.tile_pool(name="psum", bufs=3, space="PSUM") as ppool:
            rhs_sb = cpool.tile([5, N], mybir.dt.float32r)
            nc.sync.dma_start(rhs_sb, rhs.bitcast(mybir.dt.float32r))
            iota_sb = cpool.tile([128, CH], mybir.dt.float32)
            nc.sync.dma_start(iota_sb, iota)
            for blk in range(NBLK):
                lh = wpool.tile([5, 128], mybir.dt.float32r, tag="lh")
                nc.sync.dma_start(lh, lhst[blk].bitcast(mybir.dt.float32r))
                vals = rpool.tile([128, NCAND], mybir.dt.float32, tag="vals")
                for c in range(NCH):
                    t = blk * NCH + c
                    ps = ppool.tile([128, CH], mybir.dt.float32, tag="ps")
                    for h in range(CH // 512):
                        nc.tensor.matmul(
                            ps[:, h * 512:(h + 1) * 512], lh,
                            rhs_sb[:, c * CH + h * 512:c * CH + (h + 1) * 512],
                            start=True, stop=True)
                    q16 = qpool.tile([128, CH], mybir.dt.float16, tag="q16")
                    nc.scalar.activation(
                        q16, ps, mybir.ActivationFunctionType.Copy,
                        bias=0.0, scale=1.0)
                    pk = kpool.tile([128, CH], mybir.dt.float32, tag="pk")
                    if t % 7 in (1, 5):
                        nc.vector.tensor_add(out=pk, in0=q16, in1=iota_sb)
                    else:
                        nc.gpsimd.tensor_add(out=pk, in0=q16, in1=iota_sb)
                    nc.vector.max(out=vals[:, c * 8:(c + 1) * 8], in_=pk)
                nc.sync.dma_start(out_val[blk], vals)
    nc.compile()
    return nc


def _prepare_in_maps(xyz):
    sq = np.sum(xyz.astype(np.float64) ** 2, -1)
    iota_np = np.broadcast_to(
        (np.arange(CH, dtype=np.float32) * np.float32(EPS_IOTA))[None, :],
        (128, CH)).copy()
    in_maps = []
    for cid in range(NCORES):
        g0 = cid * ROWS_PER_CORE
        b, r0 = g0 // N, g0 % N
        rows = xyz[b, r0:r0 + ROWS_PER_CORE]
        lhst_np = np.empty((NBLK, 5, 128), np.float32)
        blocks = rows.reshape(NBLK, 128, 3)
        lhst_np[:, 0:3, :] = (2.0 * A) * blocks.transpose(0, 2, 1)
        lhst_np[:, 3, :] = 1.0
        lhst_np[:, 4, :] = (1024.0 - A * sq[b, r0:r0 + ROWS_PER_CORE]
                            ).reshape(NBLK, 128).astype(np.float32)
        rhs_np = np.empty((5, N), np.float32)
        rhs_np[0:3] = xyz[b].T
        rhs_np[3] = -A * sq[b]
        rhs_np[4] = 1.0
        in_maps.append({"lhst": lhst_np, "rhs": rhs_np, "iota": iota_np})
    return in_maps


def _decode_packed(pk):
    """pk: (..., ) f32 packed -> (chunk-local col idx int64, trusted bool)."""
    trusted = pk >= np.float32(512.0)
    b16 = pk.astype(np.float16)
    b32 = b16.astype(np.float32)
    over = b32 > pk
    b16f = np.where(over, np.nextafter(b16, np.float16(-np.inf)), b16
                    ).astype(np.float32)
    j = np.rint((pk - b16f) * np.float32(1.0 / EPS_IOTA)).astype(np.int64)
    return np.clip(j, 0, CH - 1), trusted


def run_device(xyz):
    """Returns cand_idx (B,N,NCAND) int64 global col indices + trusted mask."""
    from concourse import bass_utils
    nc = _build_nc()
    in_maps = _prepare_in_maps(xyz)
    res = bass_utils.run_bass_kernel_spmd(nc, in_maps,
                                          core_ids=list(range(NCORES)))
    cand = np.empty((B, N, NCAND), np.int64)
    trusted = np.empty((B, N, NCAND), bool)
    offs = (np.arange(NCH, dtype=np.int64) * CH).repeat(8)
    for cid in range(NCORES):
        g0 = cid * ROWS_PER_CORE
        b, r0 = g0 // N, g0 % N
        ov = res.results[cid]['out_val'].reshape(ROWS_PER_CORE, NCAND)
        j, tr = _decode_packed(ov)
        cand[b, r0:r0 + ROWS_PER_CORE] = j + offs[None, :]
        trusted[b, r0:r0 + ROWS_PER_CORE] = tr
    return cand, trusted


# ------------------------------------------------------------------ host ---

def refine_candidates(x, cand_idx, trusted):
    """Exact top-17 (indices+d2), bitwise == jax.lax.top_k(-d2, 17) on the
    reference's f32 d2 as computed by XLA-CPU. Rows where the candidate set
    might not contain the true top-17 are recomputed exactly in full."""
    Bn, Nn = x.shape[0], x.shape[1]
    sq = np.sum(x * x, -1)

    Y = np.take_along_axis(x[:, None, :, :], cand_idx[..., None], axis=2)
    m = np.matmul(x[:, :, None, :], Y.transpose(0, 1, 3, 2))[:, :, 0, :]
    sqc = np.take_along_axis(sq[:, None, :], cand_idx, axis=2)
    d2c = (sq[:, :, None] + sqc) - np.float32(2.0) * m
    d2c = np.where(trusted, d2c, np.float32(1e30))

    order = np.lexsort((cand_idx, d2c), axis=2)
    idx_s = np.take_along_axis(cand_idx, order, axis=2)
    d_s = np.take_along_axis(d2c, order, axis=2)
    idx17 = idx_s[:, :, :17].astype(np.int32)
    d17 = d_s[:, :, :17].copy()

    flags = np.zeros((Bn, Nn), dtype=bool)
    # self must rank first (it always should; safety)
    flags |= idx17[:, :, 0] != np.arange(Nn)[None, :]
    # duplicate candidate indices within a chunk (device anomaly)
    cg = np.sort(np.where(trusted, cand_idx, -np.arange(NCAND)[None, None, :] - 1
                          ).reshape(Bn, Nn, NCH, 8), axis=3)
    flags |= (np.diff(cg, axis=3) == 0).any(axis=(2, 3))
    # fewer than 17 trusted candidates
    flags |= d17[:, :, 16] >= np.float32(1e29)
    # window exhaustion: 17th too close to the window edge
    cand17 = d17[:, :, 16]
    flags |= cand17 >= np.float32(W - 3 * EPS_SEL)
    # chunk-insufficiency: a chunk's 8th-smallest selected d2 close enough to
    # the candidate 17th that a 9th true-top-17 element could hide in it
    # (untrusted slots -> 1e30 -> such chunks never flag; covered by the
    # window-exhaustion check instead)
    d2g = d2c.reshape(Bn, Nn, NCH, 8)
    chunk8 = d2g.max(axis=3)
    flags |= (chunk8 <= cand17[:, :, None] + np.float32(2 * EPS_SEL)).any(axis=2)

    nflag = int(flags.sum())
    if nflag:
        ar = np.arange(Nn)
        for b in range(Bn):
            rows = np.nonzero(flags[b])[0]
            if rows.size == 0:
                continue
            mrow = np.matmul(x[b][rows], x[b].T)
            d2r = (sq[b][rows][:, None] + sq[b][None, :]) - np.float32(2.0) * mrow
            o = np.lexsort((np.broadcast_to(ar, d2r.shape), d2r), axis=1)[:, :17]
            idx17[b][rows] = o.astype(np.int32)
            d17[b][rows] = np.take_along_axis(d2r, o, axis=1)
    return idx17, d17, nflag


def normals_grid_rest(xyz_np, idx17, d17):
    """Bitwise replica of the reference's post-top_k jnp stages on CPU."""
    import jax
    import jax.numpy as jnp
    cpu = jax.devices('cpu')[0]
    with jax.default_device(cpu):
        xyz = jnp.asarray(xyz_np)
        Bn = xyz.shape[0]
        K = K_NORMALS
        dists = jnp.asarray(d17)
        idx = jnp.asarray(idx17)
        nn_dists = jnp.sqrt(jnp.clip(dists[:, :, 1], 1e-8))
        nbr_idx = idx[:, :, 1:]
        neighbours = xyz[jnp.arange(Bn)[:, None, None], nbr_idx]
        centred = neighbours - xyz[:, :, None, :]
        cov = jnp.einsum('bnki,bnkj->bnij', centred, centred) / K
        _, eigvecs = jnp.linalg.eigh(cov)
        normals = eigvecs[..., :, 0]
        grid_size = jnp.clip(jnp.quantile(nn_dists, DIST_PCT / 100.0, axis=1), 1e-6)
        xyz_min = xyz.min(axis=1)
        bbox_diag = jnp.linalg.norm(xyz.max(axis=1) - xyz_min, axis=-1)
        depth_min = max(1, math.ceil(math.log2(max(NUM_GROUP + 1, 2))))
        raw_depth = jnp.log2(jnp.clip(bbox_diag / grid_size, 1.0))
        depth = jnp.clip(jnp.ceil(raw_depth).astype(jnp.int32), depth_min, 16)
        grid_coord = jnp.floor(
            (xyz - xyz_min[:, None, :]) / grid_size[:, None, None]).astype(jnp.int32)
        return (np.asarray(normals), np.asarray(grid_coord), np.asarray(depth))


def _octree_structure(normals_b, gc_b, max_depth):
    keys = {d: gc_b >> (max_depth - d) for d in range(1, max_depth + 1)}

    def mask_of(d, ix, iy, iz):
        k = keys[d]
        return (k[:, 0] == ix) & (k[:, 1] == iy) & (k[:, 2] == iz)

    def var_count(mask):
        c = int(mask.sum())
        if c < MIN_PTS:
            return 0.0, c
        n = normals_b[mask]
        m = n.mean(0)
        return float(((n - m) ** 2).sum(-1).mean()), c

    frontier = []
    for row in np.unique(keys[1], axis=0):
        ix, iy, iz = [int(v) for v in row]
        msk = mask_of(1, ix, iy, iz)
        v, c = var_count(msk)
        frontier.append(dict(d=1, ix=ix, iy=iy, iz=iz, mask=msk, variance=v,
                             count=c, splittable=(1 < max_depth and c >= MIN_PTS)))
    while len(frontier) < NUM_GROUP:
        spl = [(i, n) for i, n in enumerate(frontier) if n['splittable']]
        if not spl:
            break
        spl.sort(key=lambda t: t[1]['variance'], reverse=True)
        si, sn = spl[0]
        d, ix, iy, iz = sn['d'], sn['ix'], sn['iy'], sn['iz']
        children = []
        for dx in range(2):
            for dy in range(2):
                for dz in range(2):
                    if d + 1 > max_depth:
                        continue
                    cix, ciy, ciz = ix * 2 + dx, iy * 2 + dy, iz * 2 + dz
                    msk = mask_of(d + 1, cix, ciy, ciz)
                    if not msk.any():
                        continue
                    v, c = var_count(msk)
                    children.append(dict(d=d + 1, ix=cix, iy=ciy, iz=ciz,
                                         mask=msk, variance=v, count=c,
                                         splittable=(d + 1 < max_depth and c >= MIN_PTS)))
        frontier.pop(si)
        frontier.extend(children)
    while len(frontier) > NUM_GROUP:
        frontier.sort(key=lambda n: n['variance'])
        frontier.pop(0)
    while len(frontier) < NUM_GROUP:
        frontier.sort(key=lambda n: n['variance'], reverse=True)
        frontier.append(dict(frontier[0]))
    return [n['mask'] for n in frontier]


def group_and_normalize(xyz_np, normals_np, gc_np, depth_np):
    nbhds, ctrs = [], []
    for b in range(xyz_np.shape[0]):
        masks = _octree_structure(normals_np[b], gc_np[b], int(depth_np[b]))
        rng = np.random.default_rng(b)
        nb_b, ct_b = [], []
        for msk in masks:
            pts = xyz_np[b][msk]
            cnt = pts.shape[0]
            centroid = pts.mean(0)
            if cnt >= GROUP_SIZE:
                sel = rng.permutation(cnt)[:GROUP_SIZE]
            else:
                sel = rng.integers(0, cnt, GROUP_SIZE)
            s = pts[sel] - centroid
            scale = max(float(np.linalg.norm(s, axis=-1).max()), 1e-8)
            nb_b.append(s / scale)
            ct_b.append(centroid)
        nbhds.append(np.stack(nb_b))
        ctrs.append(np.stack(ct_b))
    neighborhood = np.stack(nbhds).astype(np.float32)
    center = np.stack(ctrs).astype(np.float32)
    return neighborhood, center


def kernel(xyz):
    xyz = np.ascontiguousarray(np.asarray(xyz), dtype=np.float32)
    assert xyz.shape == (B, N, 3), xyz.shape
    cand, trusted = run_device(xyz)
    idx17, d17, _nflag = refine_candidates(xyz, cand, trusted)
    normals, gc, depth = normals_grid_rest(xyz, idx17, d17)
    neighborhood, center = group_and_normalize(xyz, normals, gc, depth)
    return neighborhood, center


# revision 4
# speedup vs baseline: 1.5502x; 1.0023x over previous
"""AdaptiveGroup (octree point grouping) for Trainium2, 8 NeuronCores.

Pipeline:
  1. Device (Bass, SPMD x8): for every point (row), compute the affine
     KNN score p = 1024 - a*d2 against all 4096 points of its sample via
     PE matmuls (K=5 contraction, float32r), quantize to an f16 value grid
     (ACT), embed the column index in the low mantissa bits (GPSIMD/DVE
     iota add), and extract the top-8 packed scores of each 1024-column
     chunk with DVE max8. Row-parallel: core c handles 2048 of the 16384
     global rows. No index scan (max_index) is needed: the host decodes
     candidate indices from the packed f32 values.
  2. Host: exact refinement of candidates. d2 for candidates is recomputed
     with np.matmul/f32 ops that are bitwise identical to the reference's
     XLA-CPU d2; candidates are sorted by (d2, idx) which reproduces
     jax.lax.top_k exactly. Statistically-rare rows where a chunk may hide
     an extra top-17 member (or any device anomaly) are flagged and
     recomputed exactly in full.
  3. Host: the remaining reference stages (PCA normals via eigh, grid/depth,
     greedy octree budget allocation, RNG sampling + normalization) are
     data-dependent/sequential and tiny; replicated bitwise (jnp-on-CPU for
     the jnp stages, numpy for the eager stages).
"""
import math
import numpy as np

B = 4
N = 4096
NCORES = 8
ROWS_PER_CORE = B * N // NCORES   # 2048
NBLK = ROWS_PER_CORE // 128       # 16
NUM_GROUP = 128
GROUP_SIZE = 32
K_NORMALS = 16
DIST_PCT = 50.0
MIN_PTS = 4


# ---------------------------------------------------------------- device ---
# V2 packed-index design. PE matmul (K=5, float32r single-pass) computes
#   p = a*(2 x_i.x_j - sq_j) + (1024 - a*sq_i) = 1024 - a*d2
# so the trusted window d2 in [0, W] maps to p in [512, 1024]. ACT quantizes
# PSUM f32 -> f16 (value grid, step >= 0.5 in-window); GPSIMD/DVE add
# j*2^-11 (column index in the low bits); DVE max8 extracts the top-8
# packed per 1024-column chunk. Host decodes indices from the packed f32.

CH = 1024
NCH = N // CH                     # 4
NCAND = NCH * 8                   # 32 candidates per row
W = 8.0                           # d2 window; max d2_17 on this input is 5.3
A = 512.0 / W
EPS_IOTA = float(2.0 ** -11)
# Selection-noise bound in d2 units: f32r matmul error + f16 quantization +
# iota perturbation, measured <= 1.5 p-units on HW; 2x margin -> 3.0/A.
EPS_SEL = 3.0 / A


def _build_nc():
    import concourse.bacc as bacc
    import concourse.mybir as mybir
    from concourse.tile import TileContext

    nc = bacc.Bacc("TRN2", target_bir_lowering=False, debug=False,
                   num_devices=NCORES)
    lhst = nc.dram_tensor("lhst", [NBLK, 5, 128], mybir.dt.float32,
                          kind="ExternalInput").ap()
    rhs = nc.dram_tensor("rhs", [5, N], mybir.dt.float32,
                         kind="ExternalInput").ap()
    iota = nc.dram_tensor("iota", [128, CH], mybir.dt.float32,
                          kind="ExternalInput").ap()
    out_val = nc.dram_tensor("out_val", [NBLK, 128, NCAND],
                             mybir.dt.float32, kind="ExternalOutput").ap()
    with TileContext(nc) as tc:
        with tc.tile_pool(name="const", bufs=1) as cpool, \
             tc.tile_pool(name="w", bufs=3) as wpool, \
             tc.tile_pool(name="q", bufs=4) as qpool, \
             tc.tile_pool(name="pk", bufs=4) as kpool, \
             tc.tile_pool(name="res", bufs=3) as rpool, \
             tc.tile_pool(name="psum", bufs=3, space="PSUM") as ppool:
            rhs_sb = cpool.tile([5, N], mybir.dt.float32r)
            nc.sync.dma_start(rhs_sb, rhs.bitcast(mybir.dt.float32r))
            iota_sb = cpool.tile([128, CH], mybir.dt.float32)
            nc.sync.dma_start(iota_sb, iota)
            for blk in range(NBLK):
                lh = wpool.tile([5, 128], mybir.dt.float32r, tag="lh")
                nc.sync.dma_start(lh, lhst[blk].bitcast(mybir.dt.float32r))
                vals = rpool.tile([128, NCAND], mybir.dt.float32, tag="vals")
                for c in range(NCH):
                    t = blk * NCH + c
                    ps = ppool.tile([128, CH], mybir.dt.float32, tag="ps")
                    for h in range(CH // 512):
                        nc.tensor.matmul(
                            ps[:, h * 512:(h + 1) * 512], lh,
                            rhs_sb[:, c * CH + h * 512:c * CH + (h + 1) * 512],
                            start=True, stop=True)
                    q16 = qpool.tile([128, CH], mybir.dt.float16, tag="q16")
                    nc.scalar.activation(
                        q16, ps, mybir.ActivationFunctionType.Copy,
                        bias=0.0, scale=1.0)
                    pk = kpool.tile([128, CH], mybir.dt.float32, tag="pk")
                    if t % 7 in (1, 4):
                        nc.vector.tensor_add(out=pk, in0=q16, in1=iota_sb)
                    else:
                        nc.gpsimd.tensor_add(out=pk, in0=q16, in1=iota_sb)
                    nc.vector.max(out=vals[:, c * 8:(c + 1) * 8], in_=pk)
                nc.sync.dma_start(out_val[blk], vals)
    nc.compile()
    return nc


def _prepare_in_maps(xyz):
    sq = np.sum(xyz.astype(np.float64) ** 2, -1)
    iota_np = np.broadcast_to(
        (np.arange(CH, dtype=np.float32) * np.float32(EPS_IOTA))[None, :],
        (128, CH)).copy()
    in_maps = []
    for cid in range(NCORES):
        g0 = cid * ROWS_PER_CORE
        b, r0 = g0 // N, g0 % N
        rows = xyz[b, r0:r0 + ROWS_PER_CORE]
        lhst_np = np.empty((NBLK, 5, 128), np.float32)
        blocks = rows.reshape(NBLK, 128, 3)
        lhst_np[:, 0:3, :] = (2.0 * A) * blocks.transpose(0, 2, 1)
        lhst_np[:, 3, :] = 1.0
        lhst_np[:, 4, :] = (1024.0 - A * sq[b, r0:r0 + ROWS_PER_CORE]
                            ).reshape(NBLK, 128).astype(np.float32)
        rhs_np = np.empty((5, N), np.float32)
        rhs_np[0:3] = xyz[b].T
        rhs_np[3] = -A * sq[b]
        rhs_np[4] = 1.0
        in_maps.append({"lhst": lhst_np, "rhs": rhs_np, "iota": iota_np})
    return in_maps


def _decode_packed(pk):
    """pk: (..., ) f32 packed -> (chunk-local col idx int64, trusted bool)."""
    trusted = pk >= np.float32(512.0)
    b16 = pk.astype(np.float16)
    b32 = b16.astype(np.float32)
    over = b32 > pk
    b16f = np.where(over, np.nextafter(b16, np.float16(-np.inf)), b16
                    ).astype(np.float32)
    j = np.rint((pk - b16f) * np.float32(1.0 / EPS_IOTA)).astype(np.int64)
    return np.clip(j, 0, CH - 1), trusted


def run_device(xyz):
    """Returns cand_idx (B,N,NCAND) int64 global col indices + trusted mask."""
    from concourse import bass_utils
    nc = _build_nc()
    in_maps = _prepare_in_maps(xyz)
    res = bass_utils.run_bass_kernel_spmd(nc, in_maps,
                                          core_ids=list(range(NCORES)))
    cand = np.empty((B, N, NCAND), np.int64)
    trusted = np.empty((B, N, NCAND), bool)
    offs = (np.arange(NCH, dtype=np.int64) * CH).repeat(8)
    for cid in range(NCORES):
        g0 = cid * ROWS_PER_CORE
        b, r0 = g0 // N, g0 % N
        ov = res.results[cid]['out_val'].reshape(ROWS_PER_CORE, NCAND)
        j, tr = _decode_packed(ov)
        cand[b, r0:r0 + ROWS_PER_CORE] = j + offs[None, :]
        trusted[b, r0:r0 + ROWS_PER_CORE] = tr
    return cand, trusted


# ------------------------------------------------------------------ host ---

def refine_candidates(x, cand_idx, trusted):
    """Exact top-17 (indices+d2), bitwise == jax.lax.top_k(-d2, 17) on the
    reference's f32 d2 as computed by XLA-CPU. Rows where the candidate set
    might not contain the true top-17 are recomputed exactly in full."""
    Bn, Nn = x.shape[0], x.shape[1]
    sq = np.sum(x * x, -1)

    Y = np.take_along_axis(x[:, None, :, :], cand_idx[..., None], axis=2)
    m = np.matmul(x[:, :, None, :], Y.transpose(0, 1, 3, 2))[:, :, 0, :]
    sqc = np.take_along_axis(sq[:, None, :], cand_idx, axis=2)
    d2c = (sq[:, :, None] + sqc) - np.float32(2.0) * m
    d2c = np.where(trusted, d2c, np.float32(1e30))

    order = np.lexsort((cand_idx, d2c), axis=2)
    idx_s = np.take_along_axis(cand_idx, order, axis=2)
    d_s = np.take_along_axis(d2c, order, axis=2)
    idx17 = idx_s[:, :, :17].astype(np.int32)
    d17 = d_s[:, :, :17].copy()

    flags = np.zeros((Bn, Nn), dtype=bool)
    # self must rank first (it always should; safety)
    flags |= idx17[:, :, 0] != np.arange(Nn)[None, :]
    # duplicate candidate indices within a chunk (device anomaly)
    cg = np.sort(np.where(trusted, cand_idx, -np.arange(NCAND)[None, None, :] - 1
                          ).reshape(Bn, Nn, NCH, 8), axis=3)
    flags |= (np.diff(cg, axis=3) == 0).any(axis=(2, 3))
    # fewer than 17 trusted candidates
    flags |= d17[:, :, 16] >= np.float32(1e29)
    # window exhaustion: 17th too close to the window edge
    cand17 = d17[:, :, 16]
    flags |= cand17 >= np.float32(W - 3 * EPS_SEL)
    # chunk-insufficiency: a chunk's 8th-smallest selected d2 close enough to
    # the candidate 17th that a 9th true-top-17 element could hide in it
    # (untrusted slots -> 1e30 -> such chunks never flag; covered by the
    # window-exhaustion check instead)
    d2g = d2c.reshape(Bn, Nn, NCH, 8)
    chunk8 = d2g.max(axis=3)
    flags |= (chunk8 <= cand17[:, :, None] + np.float32(2 * EPS_SEL)).any(axis=2)

    nflag = int(flags.sum())
    if nflag:
        ar = np.arange(Nn)
        for b in range(Bn):
            rows = np.nonzero(flags[b])[0]
            if rows.size == 0:
                continue
            mrow = np.matmul(x[b][rows], x[b].T)
            d2r = (sq[b][rows][:, None] + sq[b][None, :]) - np.float32(2.0) * mrow
            o = np.lexsort((np.broadcast_to(ar, d2r.shape), d2r), axis=1)[:, :17]
            idx17[b][rows] = o.astype(np.int32)
            d17[b][rows] = np.take_along_axis(d2r, o, axis=1)
    return idx17, d17, nflag


def normals_grid_rest(xyz_np, idx17, d17):
    """Bitwise replica of the reference's post-top_k jnp stages on CPU."""
    import jax
    import jax.numpy as jnp
    cpu = jax.devices('cpu')[0]
    with jax.default_device(cpu):
        xyz = jnp.asarray(xyz_np)
        Bn = xyz.shape[0]
        K = K_NORMALS
        dists = jnp.asarray(d17)
        idx = jnp.asarray(idx17)
        nn_dists = jnp.sqrt(jnp.clip(dists[:, :, 1], 1e-8))
        nbr_idx = idx[:, :, 1:]
        neighbours = xyz[jnp.arange(Bn)[:, None, None], nbr_idx]
        centred = neighbours - xyz[:, :, None, :]
        cov = jnp.einsum('bnki,bnkj->bnij', centred, centred) / K
        _, eigvecs = jnp.linalg.eigh(cov)
        normals = eigvecs[..., :, 0]
        grid_size = jnp.clip(jnp.quantile(nn_dists, DIST_PCT / 100.0, axis=1), 1e-6)
        xyz_min = xyz.min(axis=1)
        bbox_diag = jnp.linalg.norm(xyz.max(axis=1) - xyz_min, axis=-1)
        depth_min = max(1, math.ceil(math.log2(max(NUM_GROUP + 1, 2))))
        raw_depth = jnp.log2(jnp.clip(bbox_diag / grid_size, 1.0))
        depth = jnp.clip(jnp.ceil(raw_depth).astype(jnp.int32), depth_min, 16)
        grid_coord = jnp.floor(
            (xyz - xyz_min[:, None, :]) / grid_size[:, None, None]).astype(jnp.int32)
        return (np.asarray(normals), np.asarray(grid_coord), np.asarray(depth))


def _octree_structure(normals_b, gc_b, max_depth):
    keys = {d: gc_b >> (max_depth - d) for d in range(1, max_depth + 1)}

    def mask_of(d, ix, iy, iz):
        k = keys[d]
        return (k[:, 0] == ix) & (k[:, 1] == iy) & (k[:, 2] == iz)

    def var_count(mask):
        c = int(mask.sum())
        if c < MIN_PTS:
            return 0.0, c
        n = normals_b[mask]
        m = n.mean(0)
        return float(((n - m) ** 2).sum(-1).mean()), c

    frontier = []
    for row in np.unique(keys[1], axis=0):
        ix, iy, iz = [int(v) for v in row]
        msk = mask_of(1, ix, iy, iz)
        v, c = var_count(msk)
        frontier.append(dict(d=1, ix=ix, iy=iy, iz=iz, mask=msk, variance=v,
                             count=c, splittable=(1 < max_depth and c >= MIN_PTS)))
    while len(frontier) < NUM_GROUP:
        spl = [(i, n) for i, n in enumerate(frontier) if n['splittable']]
        if not spl:
            break
        spl.sort(key=lambda t: t[1]['variance'], reverse=True)
        si, sn = spl[0]
        d, ix, iy, iz = sn['d'], sn['ix'], sn['iy'], sn['iz']
        children = []
        for dx in range(2):
            for dy in range(2):
                for dz in range(2):
                    if d + 1 > max_depth:
                        continue
                    cix, ciy, ciz = ix * 2 + dx, iy * 2 + dy, iz * 2 + dz
                    msk = mask_of(d + 1, cix, ciy, ciz)
                    if not msk.any():
                        continue
                    v, c = var_count(msk)
                    children.append(dict(d=d + 1, ix=cix, iy=ciy, iz=ciz,
                                         mask=msk, variance=v, count=c,
                                         splittable=(d + 1 < max_depth and c >= MIN_PTS)))
        frontier.pop(si)
        frontier.extend(children)
    while len(frontier) > NUM_GROUP:
        frontier.sort(key=lambda n: n['variance'])
        frontier.pop(0)
    while len(frontier) < NUM_GROUP:
        frontier.sort(key=lambda n: n['variance'], reverse=True)
        frontier.append(dict(frontier[0]))
    return [n['mask'] for n in frontier]


def group_and_normalize(xyz_np, normals_np, gc_np, depth_np):
    nbhds, ctrs = [], []
    for b in range(xyz_np.shape[0]):
        masks = _octree_structure(normals_np[b], gc_np[b], int(depth_np[b]))
        rng = np.random.default_rng(b)
        nb_b, ct_b = [], []
        for msk in masks:
            pts = xyz_np[b][msk]
            cnt = pts.shape[0]
            centroid = pts.mean(0)
            if cnt >= GROUP_SIZE:
                sel = rng.permutation(cnt)[:GROUP_SIZE]
            else:
                sel = rng.integers(0, cnt, GROUP_SIZE)
            s = pts[sel] - centroid
            scale = max(float(np.linalg.norm(s, axis=-1).max()), 1e-8)
            nb_b.append(s / scale)
            ct_b.append(centroid)
        nbhds.append(np.stack(nb_b))
        ctrs.append(np.stack(ct_b))
    neighborhood = np.stack(nbhds).astype(np.float32)
    center = np.stack(ctrs).astype(np.float32)
    return neighborhood, center


def kernel(xyz):
    xyz = np.ascontiguousarray(np.asarray(xyz), dtype=np.float32)
    assert xyz.shape == (B, N, 3), xyz.shape
    cand, trusted = run_device(xyz)
    idx17, d17, _nflag = refine_candidates(xyz, cand, trusted)
    normals, gc, depth = normals_grid_rest(xyz, idx17, d17)
    neighborhood, center = group_and_normalize(xyz, normals, gc, depth)
    return neighborhood, center
